# revision 31
# baseline (speedup 1.0000x reference)
"""Bass/Trainium2 kernel for nn_BayesianResNet_71408126263673.

Grouped per-sample conv: for each of 32 samples i,
  out[i] = conv2d(x[i] [128,32,32], W[i] [128oc,128c,3,3], pad=1, stride=1) + bias[i]

Sharding: b_i (32 samples) split across 8 NeuronCores, 4 samples per core.
Pure data parallel, no collectives.

Per-core kernel: each sample's conv is computed as 9 accumulating matmuls
(one per 3x3 tap) into PSUM:
  out[oc, pix] = sum_{kh,kw} W[:, :, kh, kw].T @ xpad[:, shifted pix]
with K=c=128 (partition/contraction), M=oc=128, N=512 pixels (16 output rows
per PSUM bank). The input image is zero-padded to 34x34 on the HOST so DMA
loads are fully contiguous and no memset/masking is needed on-chip. Weights
are pre-transposed on the host to [c, kh*kw, oc] so each tap is a ready-to-use
lhsT (stationary operand) tile.

Each sample's input is loaded as separately-tracked DMA chunks (weight tap
groups, image rows 0-17, image rows 16-33, with the 2-row overlap duplicated
on the host) so the matmul stream starts as soon as the first chunks land and
chases the DMA stream instead of waiting for the whole sample. Sample 0's
first-needed chunks are split across both HWDGE queues, and _hoist_startup
moves those triggers plus the PE warmup to the front of each engine's queue
(ahead of the Tile-context rendezvous) so they run during the ~6us framework
preamble. Output is stored as fp16 (converted back to fp32 on the host) to
halve store traffic, and the last sample's second block is split into two
8-row PSUM groups — with the final bias-add on the vector engine — so the
final activation+store tail is short.

Timing model per core (median ~32.6us measured, best ~31.8us):
  ~0-6.0us  fixed NEFF/queue preamble (engine-start stagger, runtime-ready
            event, two all-engine barriers, benchmark-loop register loads)
  ~6.3us    hoisted input DMAs + PE warmup start; HAM grants full PE clock
            only after ~3.5us of sustained PE activity (warmup covers it;
            if the PE idles before the grant fires, the grant defers ~5us,
            so the warmup count is sized to abut the stream start)
  ~10.6us   matmul stream starts (74 matmuls, ~216ns cadence, fp16 roofline)
  ~27us     stream ends; final bias-add + split stores + teardown

Effective per-queue DMA bandwidth under 8-core SPMD is ~60-130 GB/s (not
the 358 GB/s single-core peak), so the 2.36MB input stream is nearly
bandwidth-bound and the load schedule is deadline-ordered across the two
HWDGE queues; sample 1's arrival is the remaining (variance-bound) stall.
"""

import os
import numpy as np

import concourse.bacc as bacc
import concourse.tile as tile
from concourse import mybir
from concourse.bass_utils import run_bass_kernel_spmd

N_CORES = 8
B_I, B_J, C, H, W = 32, 1, 128, 32, 32
OC, KH, KW = 128, 3, 3
S = B_I // N_CORES            # samples per core
WP = W + 2                    # padded row width
NTAP = KH * KW                # 9
RPB = 16                      # rows per main block
IMGR = RPB + 2                # rows per image chunk (with halo)
WCOLS = NTAP * OC             # 1152 weight columns
ICOLS = IMGR * WP             # 612 image-chunk columns
XWCOLS = WCOLS + 2 * ICOLS    # 2376 per-sample columns

_DT_TABLE = {
    "fp32": (mybir.dt.float32, np.float32),
    "fp32r": (mybir.dt.float32r, np.float32),
    "fp16": (mybir.dt.float16, np.float16),
}

# Matmul operand dtype (walrus requires x and w to be both 16-bit or both
# 32-bit). Default fp16: 1 PE cycle/row with fast weight load, measured rel
# err ~3e-4 vs the fp32 reference.
_MM_DT_NAME = os.environ.get("CONV_MM_DTYPE", "fp16")
MM_DT, MM_NP = _DT_TABLE[_MM_DT_NAME]
X_DT = W_DT = MM_DT
X_NP = W_NP = MM_NP

# Output dtype: fp16 on-device (half the store bytes); host converts to fp32.
O_DT, O_NP = mybir.dt.float16, np.float16

N_WARMUP = int(os.environ.get("CONV_WARMUP", "38"))

# test.py hooks: set TRACE=True before calling kernel() to profile; the
# BassKernelResults of the last run lands in LAST_RESULTS.
TRACE = False
TRACE_KW = {}
LAST_RESULTS = None

_NC_CACHE = None


def _build_nc():
    f32 = mybir.dt.float32
    nc = bacc.Bacc()
    xw_d = nc.declare_dram_parameter("xw", [S, C, XWCOLS], MM_DT, isOutput=False)
    b_d = nc.declare_dram_parameter("b", [OC, S], f32, isOutput=False)
    o_d = nc.declare_dram_parameter("o", [S, OC, H, W], O_DT, isOutput=True)

    with tile.TileContext(nc, pool_alloc_mode="queue") as tc:
        with (
            tc.tile_pool(name="ins", bufs=1) as ins_pool,
            tc.tile_pool(name="outs", bufs=1) as outs_pool,
            tc.tile_pool(name="psum", bufs=8, space="PSUM") as psum_pool,
        ):
            # PE warmup: dependency-free matmuls on garbage data keep the
            # PE busy from engine start so the HAM clock-gate reaches 2.4 GHz
            # before the first real matmul (otherwise the first ~3.4us of
            # matmuls run at 1.2 GHz). Their PSUM tile is never read.
            wu_x = ins_pool.tile([C, OC], W_DT, tag="warmup", name="warmup")
            nc.gpsimd.memset(wu_x[:], 0.0)
            wu_ps = psum_pool.tile([C, OC], f32, name="wu_ps", tag="ps")
            for _ in range(N_WARMUP):
                nc.tensor.matmul(wu_ps[:], wu_x[:], wu_x[:], start=True, stop=True)

            WA = 3 * OC
            WIA = WCOLS + ICOLS
            # Samples 0/1 (tight deadlines) use separate w/imgA/imgB tiles so
            # their chunks can land via independent DMAs split across both
            # queues. Samples 2/3 (ample slack) use a combined [w|imgA] tile
            # loaded by ONE contiguous DMA plus an imgB tile — fewer, bigger
            # transfers cost less queue issue time and ramp better, while
            # keeping imgB a separate tile so block 0's consolidated group
            # wait doesn't cover it.
            wts = []   # [C, NTAP*OC] per sample (view for s2/s3)
            imgs = []  # ([C, IMGR, WP], [C, IMGR, WP]) per sample (views)
            wia_ts = {}
            for s in range(S):
                if s < 2:
                    wts.append(
                        ins_pool.tile([C, WCOLS], MM_DT, tag=f"w{s}", name=f"w{s}")
                    )
                    ia = ins_pool.tile(
                        [C, IMGR, WP], MM_DT, tag=f"ia{s}", name=f"ia{s}"
                    )
                else:
                    t = ins_pool.tile([C, WIA], MM_DT, tag=f"wia{s}", name=f"wia{s}")
                    wia_ts[s] = t
                    wts.append(t[:, :WCOLS])
                    ia = t[:, WCOLS:].rearrange("p (h w) -> p h w", w=WP)
                ib = ins_pool.tile([C, IMGR, WP], MM_DT, tag=f"ib{s}", name=f"ib{s}")
                imgs.append((ia, ib))
            bias_t = ins_pool.tile([OC, S], f32, tag="bias")

            # Each sample loads as 3 chunks (weights, image rows 0-17, image
            # rows 16-33). Sample 0 is split across BOTH queues (weights on
            # SP, images + bias on ACT) so its chunks land in parallel as
            # early as possible; sample 1 rides SP right behind, samples 2/3
            # are issued from the body (their deadlines are ~6-12us later).
            # The first 4 SP / 3 ACT triggers get hoisted to the front of
            # each queue by _hoist_startup.
            def load_wa(eng, s):
                eng.dma_start(wts[s][:, :WA], xw_d[s][:, :WA])

            def load_wb(eng, s):
                eng.dma_start(wts[s][:, WA:], xw_d[s][:, WA:WCOLS])

            def load_w(eng, s):
                eng.dma_start(wts[s][:], xw_d[s][:, :WCOLS])

            def load_img(eng, s, half):
                lo = WCOLS + half * ICOLS
                eng.dma_start(
                    imgs[s][half][:],
                    xw_d[s][:, lo : lo + ICOLS].rearrange("p (h w) -> p h w", w=WP),
                )

            # Sample 0's first-block needs (image rows 0-17 + weight taps 0-2,
            # 249KB) ride the SP queue alone while taps 3-8 + image rows
            # 16-33 land in parallel on ACT, so the stream can start as soon
            # as the warmup ends even with the DMA queues still ramping.
            load_img(nc.sync, 0, 0)              # hoisted
            load_wa(nc.sync, 0)                  # hoisted
            nc.scalar.dma_start(bias_t[:], b_d[:])  # hoisted
            load_wb(nc.scalar, 0)                # hoisted
            load_img(nc.scalar, 0, 1)            # hoisted
            # Body-issued loads, deadline order. Effective per-queue DMA
            # bandwidth is ~120 GB/s before the matmul stream starts but
            # collapses to ~40-75 GB/s once the PE is streaming (SBUF port
            # contention), so sample 1 — whose deadline is only ~4us after
            # stream start — keeps w+imgA on SP with imgB on ACT, while
            # samples 2/3 load as one whole-sample DMA each.
            load_w(nc.sync, 1)
            load_img(nc.sync, 1, 0)
            load_img(nc.scalar, 1, 1)
            nc.scalar.dma_start(wia_ts[2][:], xw_d[2][:, :WIA])
            load_img(nc.scalar, 2, 1)
            nc.sync.dma_start(wia_ts[3][:], xw_d[3][:, :WIA])
            load_img(nc.sync, 3, 1)

            def conv_block(s, half, row0, nrows, ps_name):
                """One accumulation group: output rows [row0, row0+nrows) with
                row0 relative to image chunk `half` (global row0 = row0 + 16*half)."""
                ps = psum_pool.tile([OC, nrows, W], f32, name=ps_name, tag="ps")
                img = imgs[s][half]
                for t in range(NTAP):
                    kh, kw = divmod(t, KW)
                    rhs = img[:, row0 + kh : row0 + kh + nrows, kw : kw + W]
                    lhsT = wts[s][:, t * OC : (t + 1) * OC]
                    nc.tensor.matmul(
                        ps[:], lhsT, rhs, start=(t == 0), stop=(t == NTAP - 1)
                    )
                return ps

            for s in range(S):
                out_t = outs_pool.tile([OC, H, W], O_DT, tag=f"out{s}", name=f"out{s}")
                if s < S - 1:
                    # (half, local row0, nrows, global row0)
                    blocks = [(0, 0, RPB, 0), (1, 0, RPB, RPB)]
                else:
                    # last sample: split block 1 so the final activation +
                    # store cover only 8 rows each.
                    blocks = [(0, 0, RPB, 0), (1, 0, 8, RPB), (1, 8, 8, RPB + 8)]
                for bi, (half, lr0, nrows, gr0) in enumerate(blocks):
                    ps = conv_block(s, half, lr0, nrows, f"ps{s}_{bi}")
                    if s == S - 1 and bi == len(blocks) - 1:
                        # The very last bias-add runs on the (otherwise idle)
                        # vector engine: the scalar engine is still issuing
                        # the previous block's store trigger when this PSUM
                        # group completes, and this block gates the final
                        # store.
                        nc.vector.tensor_scalar_add(
                            out_t[:, gr0 : gr0 + nrows, :],
                            ps[:],
                            bias_t[:, s : s + 1],
                        )
                    else:
                        nc.scalar.activation(
                            out_t[:, gr0 : gr0 + nrows, :],
                            ps[:],
                            mybir.ActivationFunctionType.Identity,
                            bias=bias_t[:, s : s + 1],
                        )
                    if s == S - 1:
                        # Stream the last sample's output per block on
                        # alternating queues so the tail is short; the final
                        # block is split across both queues so its two 4-row
                        # halves transfer in parallel.
                        if bi == len(blocks) - 1:
                            if os.environ.get("CONV_FINAL_STORE", "split") == "gpsimd":
                                # Software-DGE path on the otherwise-idle
                                # gpsimd: lower small-transfer latency than a
                                # HWDGE trigger (~0.6us) + flight (~1.2us).
                                nc.gpsimd.dma_start(
                                    o_d[s][:, gr0 : gr0 + nrows, :],
                                    out_t[:, gr0 : gr0 + nrows, :],
                                )
                            else:
                                hr = nrows // 2
                                nc.sync.dma_start(
                                    o_d[s][:, gr0 : gr0 + hr, :],
                                    out_t[:, gr0 : gr0 + hr, :],
                                )
                                nc.scalar.dma_start(
                                    o_d[s][:, gr0 + hr : gr0 + nrows, :],
                                    out_t[:, gr0 + hr : gr0 + nrows, :],
                                )
                        else:
                            eng = nc.scalar if bi == 0 else nc.sync
                            eng.dma_start(
                                o_d[s][:, gr0 : gr0 + nrows, :],
                                out_t[:, gr0 : gr0 + nrows, :],
                            )
                if s < S - 1:
                    # s0's store rides SP (ACT still needs its remaining input
                    # budget for s2's deadline); s1 -> ACT (done loading by
                    # then); s2 -> SP.
                    eng = nc.scalar if s == 1 else nc.sync
                    eng.dma_start(o_d[s], out_t[:])
    nc.compile()
    _hoist_startup(nc)
    return nc


# Post-compile IR surgery: the Tile body only starts executing ~7us into the
# kernel (engine-start stagger, a runtime-ready event wait on the PE queue,
# two all-engine barriers, benchmark-loop register loads, and a rendezvous).
# The input DMA triggers and the PE warmup don't depend on any of that, so
# hoist them to the very front of each engine's queue (before the preamble
# InstCall): the DMAs stream while the preamble runs and sample 0 is resident
# in SBUF by the time the matmul stream can start; the warmup matmuls run
# during the PE queue's runtime-ready wait and claim the HAM full-clock grant
# early.
N_HOIST_SP = int(os.environ.get("CONV_HOIST_SP", "2"))    # s0 imgA + w012
N_HOIST_ACT = int(os.environ.get("CONV_HOIST_ACT", "3"))  # bias + s0 imgs
N_PRE_WARMUP = min(int(os.environ.get("CONV_PRE_WARMUP", str(N_WARMUP))), N_WARMUP)


def _hoist_startup(nc):
    f = nc.m.functions[0]
    main = f.blocks[0]
    body = next(b for b in f.blocks if b.name.startswith("tile_context"))
    E = mybir.EngineType

    hoisted = []

    def take(pred, n):
        got = [i for i in body.instructions if pred(i)][:n]
        for i in got:
            body.instructions.remove(i)
        hoisted.extend(got)

    # Warmup memset (first Pool memset) + first N warmup ldweights/matmul
    # pairs. The first ldweights waits on the memset's semaphore, so the
    # memset must move whenever any warmup pair moves.
    if N_PRE_WARMUP > 0:
        take(lambda i: i.engine == E.Pool and type(i).__name__ == "InstMemset", 1)
        take(
            lambda i: i.engine == E.PE
            and type(i).__name__ in ("InstLdweights", "InstMatmult"),
            2 * N_PRE_WARMUP,
        )
    take(
        lambda i: i.engine == E.SP and type(i).__name__ == "InstDMACopy",
        N_HOIST_SP,
    )
    take(
        lambda i: i.engine == E.Activation and type(i).__name__ == "InstDMACopy",
        N_HOIST_ACT,
    )

    for inst in reversed(hoisted):
        main.instructions.insert(0, inst)


def _get_nc():
    global _NC_CACHE
    if _NC_CACHE is None:
        _NC_CACHE = _build_nc()
    return _NC_CACHE


def kernel(x: np.ndarray, weight: np.ndarray, bias: np.ndarray) -> np.ndarray:
    global LAST_RESULTS
    assert x.shape == (B_I, B_J, C, H, W)
    assert weight.shape == (B_I, OC, C, KH, KW)
    assert bias.shape == (B_I, B_J, OC)

    x = np.asarray(x, dtype=np.float32)
    weight = np.asarray(weight, dtype=np.float32)
    bias = np.asarray(bias, dtype=np.float32)

    # Host-side layout prep (part of sharding): zero-pad images, transpose
    # weights so each 3x3 tap is a contiguous [c, oc] stationary tile.
    # Per-sample buffer: [weights 1152 | img rows 0-17 | img rows 16-33]
    # with the 2-row halo overlap duplicated.
    xw = np.zeros((B_I, C, XWCOLS), dtype=MM_NP)
    wt = np.ascontiguousarray(weight.transpose(0, 2, 3, 4, 1))  # [b_i, c, kh, kw, oc]
    xw[:, :, :WCOLS] = wt.reshape(B_I, C, WCOLS).astype(MM_NP)
    x16 = x[:, 0].astype(MM_NP)  # [b_i, c, 32, 32]
    for half in range(2):
        chunk = xw[
            :, :, WCOLS + half * ICOLS : WCOLS + (half + 1) * ICOLS
        ].reshape(B_I, C, IMGR, WP)
        r0 = half * RPB  # global padded-row start of this chunk
        # padded rows r0 .. r0+17 ; padded row p holds x row p-1 for 1<=p<=32
        xr0 = max(r0, 1) - 1
        xr1 = min(r0 + IMGR, H + 1) - 1
        chunk[:, :, max(r0, 1) - r0 : xr1 + 1 - r0, 1 : 1 + W] = x16[:, :, xr0:xr1]
    bt = bias[:, 0, :]  # [b_i, oc]

    in_maps = []
    for core in range(N_CORES):
        sl = slice(core * S, (core + 1) * S)
        in_maps.append(
            {
                "xw": np.ascontiguousarray(xw[sl]),
                "b": np.ascontiguousarray(bt[sl].T),  # [OC, S]
            }
        )

    nc = _get_nc()
    try:
        res = run_bass_kernel_spmd(
            nc, in_maps, core_ids=list(range(N_CORES)), trace=TRACE, **TRACE_KW
        )
    except Exception:
        # Transient NRT/device errors (e.g. NRT_EXEC_UNIT_UNRECOVERABLE after
        # heavy reuse) usually clear on retry; the work is idempotent.
        import time

        time.sleep(10)
        res = run_bass_kernel_spmd(
            nc, in_maps, core_ids=list(range(N_CORES)), trace=TRACE, **TRACE_KW
        )
    LAST_RESULTS = res

    out = np.concatenate([res.results[c]["o"] for c in range(N_CORES)], axis=0)
    return out.reshape(B_I, B_J, OC, H, W).astype(np.float32)


# revision 32
# speedup vs baseline: 1.0608x; 1.0608x over previous
"""Bass/Trainium2 kernel for nn_BayesianResNet_71408126263673.

Grouped per-sample conv: for each of 32 samples i,
  out[i] = conv2d(x[i] [128,32,32], W[i] [128oc,128c,3,3], pad=1, stride=1) + bias[i]

Sharding: b_i (32 samples) split across 8 NeuronCores, 4 samples per core.
Pure data parallel, no collectives.

Per-core kernel: each sample's conv is computed as 9 accumulating matmuls
(one per 3x3 tap) into PSUM:
  out[oc, pix] = sum_{kh,kw} W[:, :, kh, kw].T @ xpad[:, shifted pix]
with K=c=128 (partition/contraction), M=oc=128, N=512 pixels (16 output rows
per PSUM bank). The input image is zero-padded to 34x34 on the HOST so DMA
loads are fully contiguous and no memset/masking is needed on-chip. Weights
are pre-transposed on the host to [c, kh*kw, oc] so each tap is a ready-to-use
lhsT (stationary operand) tile.

Each sample's input is loaded as separately-tracked DMA chunks (weight tap
groups, image rows 0-17, image rows 16-33, with the 2-row overlap duplicated
on the host) so the matmul stream starts as soon as the first chunks land and
chases the DMA stream instead of waiting for the whole sample. Sample 0's
first-needed chunks are split across both HWDGE queues, and _hoist_startup
moves those triggers plus the PE warmup to the front of each engine's queue
(ahead of the Tile-context rendezvous) so they run during the ~6us framework
preamble. Output is stored as fp16 (converted back to fp32 on the host) to
halve store traffic, and the last sample's second block is split into two
8-row PSUM groups — with the final bias-add on the vector engine — so the
final activation+store tail is short.

Timing model per core (median ~32.6us measured, best ~31.8us):
  ~0-6.0us  fixed NEFF/queue preamble (engine-start stagger, runtime-ready
            event, two all-engine barriers, benchmark-loop register loads)
  ~6.3us    hoisted input DMAs + PE warmup start; HAM grants full PE clock
            only after ~3.5us of sustained PE activity (warmup covers it;
            if the PE idles before the grant fires, the grant defers ~5us,
            so the warmup count is sized to abut the stream start)
  ~10.6us   matmul stream starts (74 matmuls, ~216ns cadence, fp16 roofline)
  ~27us     stream ends; final bias-add + split stores + teardown

Effective per-queue DMA bandwidth under 8-core SPMD is ~60-130 GB/s (not
the 358 GB/s single-core peak), so the 2.36MB input stream is nearly
bandwidth-bound and the load schedule is deadline-ordered across the two
HWDGE queues; sample 1's arrival is the remaining (variance-bound) stall.
"""

import os
import numpy as np

import concourse.bacc as bacc
import concourse.tile as tile
from concourse import mybir
from concourse.bass_utils import run_bass_kernel_spmd

N_CORES = 8
B_I, B_J, C, H, W = 32, 1, 128, 32, 32
OC, KH, KW = 128, 3, 3
S = B_I // N_CORES            # samples per core
WP = W + 2                    # padded row width
NTAP = KH * KW                # 9
RPB = 16                      # rows per main block
IMGR = RPB + 2                # rows per image chunk (with halo)
WCOLS = NTAP * OC             # 1152 weight columns
ICOLS = IMGR * WP             # 612 image-chunk columns
XWCOLS = WCOLS + 2 * ICOLS    # 2376 per-sample columns

_DT_TABLE = {
    "fp32": (mybir.dt.float32, np.float32),
    "fp32r": (mybir.dt.float32r, np.float32),
    "fp16": (mybir.dt.float16, np.float16),
}

# Matmul operand dtype (walrus requires x and w to be both 16-bit or both
# 32-bit). Default fp16: 1 PE cycle/row with fast weight load, measured rel
# err ~3e-4 vs the fp32 reference.
_MM_DT_NAME = os.environ.get("CONV_MM_DTYPE", "fp16")
MM_DT, MM_NP = _DT_TABLE[_MM_DT_NAME]
X_DT = W_DT = MM_DT
X_NP = W_NP = MM_NP

# Output dtype: fp16 on-device (half the store bytes); host converts to fp32.
O_DT, O_NP = mybir.dt.float16, np.float16

N_WARMUP = int(os.environ.get("CONV_WARMUP", "38"))

# test.py hooks: set TRACE=True before calling kernel() to profile; the
# BassKernelResults of the last run lands in LAST_RESULTS.
TRACE = False
TRACE_KW = {}
LAST_RESULTS = None

_NC_CACHE = None


def _build_nc():
    f32 = mybir.dt.float32
    nc = bacc.Bacc()
    xw_d = nc.declare_dram_parameter("xw", [S, C, XWCOLS], MM_DT, isOutput=False)
    b_d = nc.declare_dram_parameter("b", [OC, S], f32, isOutput=False)
    o_d = nc.declare_dram_parameter("o", [S, OC, H, W], O_DT, isOutput=True)

    with tile.TileContext(nc, pool_alloc_mode="queue") as tc:
        with (
            tc.tile_pool(name="ins", bufs=1) as ins_pool,
            tc.tile_pool(name="outs", bufs=1) as outs_pool,
            tc.tile_pool(name="psum", bufs=8, space="PSUM") as psum_pool,
        ):
            # PE warmup: dependency-free matmuls on garbage data keep the
            # PE busy from engine start so the HAM clock-gate reaches 2.4 GHz
            # before the first real matmul (otherwise the first ~3.4us of
            # matmuls run at 1.2 GHz). Their PSUM tile is never read.
            wu_x = ins_pool.tile([C, OC], W_DT, tag="warmup", name="warmup")
            nc.gpsimd.memset(wu_x[:], 0.0)
            wu_ps = psum_pool.tile([C, OC], f32, name="wu_ps", tag="ps")
            for _ in range(N_WARMUP):
                nc.tensor.matmul(wu_ps[:], wu_x[:], wu_x[:], start=True, stop=True)

            WA = 3 * OC
            wts = []   # [C, NTAP*OC] per sample
            imgs = []  # ([C, IMGR, WP], [C, IMGR, WP]) per sample
            for s in range(S):
                wts.append(
                    ins_pool.tile([C, WCOLS], MM_DT, tag=f"w{s}", name=f"w{s}")
                )
                ia = ins_pool.tile([C, IMGR, WP], MM_DT, tag=f"ia{s}", name=f"ia{s}")
                ib = ins_pool.tile([C, IMGR, WP], MM_DT, tag=f"ib{s}", name=f"ib{s}")
                imgs.append((ia, ib))
            bias_t = ins_pool.tile([OC, S], f32, tag="bias")

            # Each sample loads as 3 chunks (weights, image rows 0-17, image
            # rows 16-33). Sample 0 is split across BOTH queues (weights on
            # SP, images + bias on ACT) so its chunks land in parallel as
            # early as possible; sample 1 rides SP right behind, samples 2/3
            # are issued from the body (their deadlines are ~6-12us later).
            # The first 4 SP / 3 ACT triggers get hoisted to the front of
            # each queue by _hoist_startup.
            def load_wa(eng, s):
                eng.dma_start(wts[s][:, :WA], xw_d[s][:, :WA])

            def load_wb(eng, s):
                eng.dma_start(wts[s][:, WA:], xw_d[s][:, WA:WCOLS])

            def load_w(eng, s):
                eng.dma_start(wts[s][:], xw_d[s][:, :WCOLS])

            def load_img(eng, s, half):
                lo = WCOLS + half * ICOLS
                eng.dma_start(
                    imgs[s][half][:],
                    xw_d[s][:, lo : lo + ICOLS].rearrange("p (h w) -> p h w", w=WP),
                )

            # Sample 0's first-block needs (image rows 0-17 + weight taps 0-2,
            # 249KB) ride the SP queue alone while taps 3-8 + image rows
            # 16-33 land in parallel on ACT, so the stream can start as soon
            # as the warmup ends even with the DMA queues still ramping.
            load_img(nc.sync, 0, 0)              # hoisted
            load_wa(nc.sync, 0)                  # hoisted
            nc.scalar.dma_start(bias_t[:], b_d[:])  # hoisted
            load_wb(nc.scalar, 0)                # hoisted
            load_img(nc.scalar, 0, 1)            # hoisted
            # Body-issued loads, deadline order. Effective per-queue DMA
            # bandwidth is ~120 GB/s before the matmul stream starts but
            # collapses to ~40-75 GB/s once the PE is streaming (SBUF port
            # contention), so sample 1 — whose deadline is only ~4us after
            # stream start — is split across both queues like sample 0.
            def load_w_range(eng, s, t0, t1):
                eng.dma_start(
                    wts[s][:, t0 * OC : t1 * OC], xw_d[s][:, t0 * OC : t1 * OC]
                )

            load_w_range(nc.sync, 1, 0, 5)
            load_img(nc.sync, 1, 0)
            load_w_range(nc.scalar, 1, 5, NTAP)
            load_img(nc.scalar, 1, 1)
            load_w(nc.scalar, 2)
            load_img(nc.scalar, 2, 0)
            load_img(nc.scalar, 2, 1)
            load_w(nc.sync, 3)
            load_img(nc.sync, 3, 0)
            load_img(nc.sync, 3, 1)

            def conv_block(s, half, row0, nrows, ps_name):
                """One accumulation group: output rows [row0, row0+nrows) with
                row0 relative to image chunk `half` (global row0 = row0 + 16*half)."""
                ps = psum_pool.tile([OC, nrows, W], f32, name=ps_name, tag="ps")
                img = imgs[s][half]
                for t in range(NTAP):
                    kh, kw = divmod(t, KW)
                    rhs = img[:, row0 + kh : row0 + kh + nrows, kw : kw + W]
                    lhsT = wts[s][:, t * OC : (t + 1) * OC]
                    nc.tensor.matmul(
                        ps[:], lhsT, rhs, start=(t == 0), stop=(t == NTAP - 1)
                    )
                return ps

            for s in range(S):
                out_t = outs_pool.tile([OC, H, W], O_DT, tag=f"out{s}", name=f"out{s}")
                if s < S - 1:
                    # (half, local row0, nrows, global row0)
                    blocks = [(0, 0, RPB, 0), (1, 0, RPB, RPB)]
                else:
                    # last sample: split block 1 so the final activation +
                    # store cover only 8 rows each.
                    blocks = [(0, 0, RPB, 0), (1, 0, 8, RPB), (1, 8, 8, RPB + 8)]
                for bi, (half, lr0, nrows, gr0) in enumerate(blocks):
                    ps = conv_block(s, half, lr0, nrows, f"ps{s}_{bi}")
                    if s == S - 1 and bi == len(blocks) - 1:
                        # The very last bias-add runs on the (otherwise idle)
                        # vector engine: the scalar engine is still issuing
                        # the previous block's store trigger when this PSUM
                        # group completes, and this block gates the final
                        # store.
                        nc.vector.tensor_scalar_add(
                            out_t[:, gr0 : gr0 + nrows, :],
                            ps[:],
                            bias_t[:, s : s + 1],
                        )
                    else:
                        nc.scalar.activation(
                            out_t[:, gr0 : gr0 + nrows, :],
                            ps[:],
                            mybir.ActivationFunctionType.Identity,
                            bias=bias_t[:, s : s + 1],
                        )
                    if s == S - 1:
                        # Stream the last sample's output per block on
                        # alternating queues so the tail is short; the final
                        # block is split across both queues so its two 4-row
                        # halves transfer in parallel.
                        if bi == len(blocks) - 1:
                            if os.environ.get("CONV_FINAL_STORE", "split") == "gpsimd":
                                # Software-DGE path on the otherwise-idle
                                # gpsimd: lower small-transfer latency than a
                                # HWDGE trigger (~0.6us) + flight (~1.2us).
                                nc.gpsimd.dma_start(
                                    o_d[s][:, gr0 : gr0 + nrows, :],
                                    out_t[:, gr0 : gr0 + nrows, :],
                                )
                            else:
                                hr = nrows // 2
                                nc.sync.dma_start(
                                    o_d[s][:, gr0 : gr0 + hr, :],
                                    out_t[:, gr0 : gr0 + hr, :],
                                )
                                nc.scalar.dma_start(
                                    o_d[s][:, gr0 + hr : gr0 + nrows, :],
                                    out_t[:, gr0 + hr : gr0 + nrows, :],
                                )
                        else:
                            eng = nc.scalar if bi == 0 else nc.sync
                            eng.dma_start(
                                o_d[s][:, gr0 : gr0 + nrows, :],
                                out_t[:, gr0 : gr0 + nrows, :],
                            )
                if s < S - 1:
                    # s0's store rides SP (ACT still needs its remaining input
                    # budget for s2's deadline); s1 -> ACT (done loading by
                    # then); s2 -> SP.
                    eng = nc.scalar if s == 1 else nc.sync
                    eng.dma_start(o_d[s], out_t[:])
    nc.compile()
    _hoist_startup(nc)
    return nc


# Post-compile IR surgery: the Tile body only starts executing ~7us into the
# kernel (engine-start stagger, a runtime-ready event wait on the PE queue,
# two all-engine barriers, benchmark-loop register loads, and a rendezvous).
# The input DMA triggers and the PE warmup don't depend on any of that, so
# hoist them to the very front of each engine's queue (before the preamble
# InstCall): the DMAs stream while the preamble runs and sample 0 is resident
# in SBUF by the time the matmul stream can start; the warmup matmuls run
# during the PE queue's runtime-ready wait and claim the HAM full-clock grant
# early.
N_HOIST_SP = int(os.environ.get("CONV_HOIST_SP", "2"))    # s0 imgA + w012
N_HOIST_ACT = int(os.environ.get("CONV_HOIST_ACT", "3"))  # bias + s0 imgs
N_PRE_WARMUP = min(int(os.environ.get("CONV_PRE_WARMUP", str(N_WARMUP))), N_WARMUP)


def _hoist_startup(nc):
    f = nc.m.functions[0]
    main = f.blocks[0]
    body = next(b for b in f.blocks if b.name.startswith("tile_context"))
    E = mybir.EngineType

    hoisted = []

    def take(pred, n):
        got = [i for i in body.instructions if pred(i)][:n]
        for i in got:
            body.instructions.remove(i)
        hoisted.extend(got)

    # Warmup memset (first Pool memset) + first N warmup ldweights/matmul
    # pairs. The first ldweights waits on the memset's semaphore, so the
    # memset must move whenever any warmup pair moves.
    if N_PRE_WARMUP > 0:
        take(lambda i: i.engine == E.Pool and type(i).__name__ == "InstMemset", 1)
        take(
            lambda i: i.engine == E.PE
            and type(i).__name__ in ("InstLdweights", "InstMatmult"),
            2 * N_PRE_WARMUP,
        )
    take(
        lambda i: i.engine == E.SP and type(i).__name__ == "InstDMACopy",
        N_HOIST_SP,
    )
    take(
        lambda i: i.engine == E.Activation and type(i).__name__ == "InstDMACopy",
        N_HOIST_ACT,
    )

    for inst in reversed(hoisted):
        main.instructions.insert(0, inst)


def _get_nc():
    global _NC_CACHE
    if _NC_CACHE is None:
        _NC_CACHE = _build_nc()
    return _NC_CACHE


def kernel(x: np.ndarray, weight: np.ndarray, bias: np.ndarray) -> np.ndarray:
    global LAST_RESULTS
    assert x.shape == (B_I, B_J, C, H, W)
    assert weight.shape == (B_I, OC, C, KH, KW)
    assert bias.shape == (B_I, B_J, OC)

    x = np.asarray(x, dtype=np.float32)
    weight = np.asarray(weight, dtype=np.float32)
    bias = np.asarray(bias, dtype=np.float32)

    # Host-side layout prep (part of sharding): zero-pad images, transpose
    # weights so each 3x3 tap is a contiguous [c, oc] stationary tile.
    # Per-sample buffer: [weights 1152 | img rows 0-17 | img rows 16-33]
    # with the 2-row halo overlap duplicated.
    xw = np.zeros((B_I, C, XWCOLS), dtype=MM_NP)
    wt = np.ascontiguousarray(weight.transpose(0, 2, 3, 4, 1))  # [b_i, c, kh, kw, oc]
    xw[:, :, :WCOLS] = wt.reshape(B_I, C, WCOLS).astype(MM_NP)
    x16 = x[:, 0].astype(MM_NP)  # [b_i, c, 32, 32]
    for half in range(2):
        chunk = xw[
            :, :, WCOLS + half * ICOLS : WCOLS + (half + 1) * ICOLS
        ].reshape(B_I, C, IMGR, WP)
        r0 = half * RPB  # global padded-row start of this chunk
        # padded rows r0 .. r0+17 ; padded row p holds x row p-1 for 1<=p<=32
        xr0 = max(r0, 1) - 1
        xr1 = min(r0 + IMGR, H + 1) - 1
        chunk[:, :, max(r0, 1) - r0 : xr1 + 1 - r0, 1 : 1 + W] = x16[:, :, xr0:xr1]
    bt = bias[:, 0, :]  # [b_i, oc]

    in_maps = []
    for core in range(N_CORES):
        sl = slice(core * S, (core + 1) * S)
        in_maps.append(
            {
                "xw": np.ascontiguousarray(xw[sl]),
                "b": np.ascontiguousarray(bt[sl].T),  # [OC, S]
            }
        )

    nc = _get_nc()
    try:
        res = run_bass_kernel_spmd(
            nc, in_maps, core_ids=list(range(N_CORES)), trace=TRACE, **TRACE_KW
        )
    except Exception:
        # Transient NRT/device errors (e.g. NRT_EXEC_UNIT_UNRECOVERABLE after
        # heavy reuse) usually clear on retry; the work is idempotent.
        import time

        time.sleep(10)
        res = run_bass_kernel_spmd(
            nc, in_maps, core_ids=list(range(N_CORES)), trace=TRACE, **TRACE_KW
        )
    LAST_RESULTS = res

    out = np.concatenate([res.results[c]["o"] for c in range(N_CORES)], axis=0)
    return out.reshape(B_I, B_J, OC, H, W).astype(np.float32)
